# revision 31
# baseline (speedup 1.0000x reference)
"""Trainium2 Bass kernel for nn_AttentionLayer (DIN-style attention scoring MLP).

Math (per batch b, key position s):
    feats = [q, k, q*k, q-k]                       # [4E] = 256
    h1 = relu(feats @ W0 + b0)                     # 128
    h2 = relu(h1 @ W1 + b1)                        # 64
    score = h2 @ W2 + b2                           # scalar
    attn = softmax_s(score masked to s < len[b])
    out = sum_s attn[s] * k[s]                     # [E]

Host-side algebra (exact):
    W0 rows: [0:64]=Wq(q), [64:128]=Wkk(k), [128:192]=Wc(q*k), [192:256]=Wd(q-k)
    h1 = relu(q@(Wq+Wd) + k@(Wkk-Wd) + (q*k)@Wc + b0)
  Invalid key columns (s >= len[b]) are zeroed on the host, so the device's
  unnormalized contraction sum_s exp(score)*k skips them exactly; the softmax
  normalization (divide by masked sum) happens on the host during the gather.

Device layout: feature-major ("transposed") tiles, batch-pair packing.
  Each pair p handles 8 batches = 2 groups (A=batches 8p..8p+3 in
  partitions 0..63, B=batches 8p+4..8p+7 in partitions 64..127).
  N = 4 batches * 100 positions = 400 columns per group.
"""

import numpy as np

B, S, E = 8192, 100, 64
H1, H2 = 128, 64
NCORES = 8
BC = B // NCORES          # 1024 batches per core
TB = 4                    # batches per group
N = TB * S                # 400 columns per matmul
NP = BC // (2 * TB)       # 128 pairs per core

MM_DTYPE = "bfloat16"     # "bfloat16" | "float32r" | "float32"

_PROG = {}


def _build_program():
    import concourse.bacc as bacc
    import concourse.tile as tile
    import concourse.mybir as mybir

    f32 = mybir.dt.float32
    dt_mm = getattr(mybir.dt, MM_DTYPE)
    nc = bacc.Bacc(
        "TRN2", target_bir_lowering=False, debug=False, num_devices=NCORES)

    # ---- DRAM I/O ----
    # kTP pair-packed: [128, NP*N]; rows 0:64 = keys^T of group 2p,
    # rows 64:128 = keys^T of group 2p+1, at columns p*N:(p+1)*N.
    kT_d = nc.declare_dram_parameter("kTP", [128, NP * N], dt_mm, isOutput=False)
    kTF_d = nc.declare_dram_parameter("kTF", [128, NP * N], f32, isOutput=False)
    ql_d = nc.declare_dram_parameter("qlP", [NP, 128, TB], dt_mm, isOutput=False)
    # weights packed [128, 3*H1 + 2*H2] in matmul dtype
    NW = 3 * H1 + 2 * H2
    w_d = nc.declare_dram_parameter("wpack", [128, NW], dt_mm, isOutput=False)
    b_d = nc.declare_dram_parameter("bpack", [128, 3], f32, isOutput=False)

    em_d = nc.declare_dram_parameter("em2", [2 * NP, N], f32, isOutput=True)
    outT_d = nc.declare_dram_parameter("outT", [128, BC // 2], f32, isOutput=True)

    AF = mybir.ActivationFunctionType
    OP = mybir.AluOpType

    with tile.TileContext(nc) as tc:
        with (
            tc.tile_pool(name="consts", bufs=1) as cpool,
            tc.tile_pool(name="kin", bufs=4) as kpool,
            tc.tile_pool(name="qlp", bufs=4) as qlpool,
            tc.tile_pool(name="mid", bufs=4) as mpool,
            tc.tile_pool(name="psl0", bufs=2, space="PSUM") as psl0pool,
            tc.tile_pool(name="psh2", bufs=2, space="PSUM") as psh2pool,
            tc.tile_pool(name="pss", bufs=2, space="PSUM") as psspool,
            tc.tile_pool(name="acc", bufs=1) as apool,
        ):
            w_t = cpool.tile([128, NW], dt_mm, tag="wpack")
            b_t = cpool.tile([128, 3], f32, tag="bpack")
            nc.sync.dma_start(w_t[:], w_d[:])
            nc.sync.dma_start(b_t[:], b_d[:])
            wq_t = w_t[:, 0:H1]
            wk_t = w_t[:, H1:2 * H1]
            wc_t = w_t[:, 2 * H1:3 * H1]
            w1_t = w_t[:, 3 * H1:3 * H1 + H2]
            w2_t = w_t[:, 3 * H1 + H2:3 * H1 + 2 * H2]
            b0_t = b_t[:, 0:1]
            b1_t = b_t[:, 1:2]
            b2_t = b_t[:, 2:3]

            outT_t = apool.tile([128, BC // 2], f32, tag="outT")

            KB = min(4, NP)    # pairs per keys-block DMA / qk op
            QB = min(8, NP)    # pairs per ql block DMA
            ktb = None
            qkb = None
            qlb = None
            for p in range(NP):
                # ---- block loads: keys (4 pairs), q (8 pairs) ----
                if p % QB == 0:
                    qlb = qlpool.tile([128, QB * TB], dt_mm, tag="ql")
                    nc.sync.dma_start(
                        qlb[:].rearrange("p (q t) -> p q t", q=QB),
                        ql_d[p:p + QB].rearrange("q p t -> p q t"))
                if p % KB == 0:
                    ktb = kpool.tile([128, KB * N], dt_mm, tag="kt")
                    nc.sync.dma_start(ktb[:], kT_d[:, p * N:(p + KB) * N])
                    ktfb = kpool.tile([128, KB * N], f32, tag="ktf")
                    nc.sync.dma_start(ktfb[:], kTF_d[:, p * N:(p + KB) * N])
                    qkb = kpool.tile([128, KB * N], dt_mm, tag="qk")
                    q_bc = (qlb[:, (p % QB) * TB:(p % QB + KB) * TB]
                            .unsqueeze(2).broadcast_to((128, KB * TB, S)))
                    nc.gpsimd.tensor_tensor(
                        qkb[:].rearrange("p (b s) -> p b s", s=S),
                        ktb[:].rearrange("p (b s) -> p b s", s=S),
                        q_bc,
                        op=OP.mult,
                    )
                kt = ktb[:, (p % KB) * N:(p % KB + 1) * N]
                ktf = ktfb[:, (p % KB) * N:(p % KB + 1) * N]
                qk = qkb[:, (p % KB) * N:(p % KB + 1) * N]
                ql_t = qlb[:, (p % QB) * TB:(p % QB + 1) * TB]

                # ---- layer 0: h1 = relu(Wk.T k + Wc.T qk + Wq.T q + b0) ----
                ps0 = psl0pool.tile([128, 1024], f32, tag="ps0")
                qa_bc = ql_t[0:E].unsqueeze(2).broadcast_to((E, TB, S))
                qb_bc = ql_t[E:128].unsqueeze(2).broadcast_to((E, TB, S))
                nc.tensor.matmul(ps0[:, 0:N], wk_t[0:E, :], kt[0:E],
                                 start=True, stop=False)
                nc.tensor.matmul(ps0[:, 0:N], wc_t[0:E, :], qk[0:E],
                                 start=False, stop=False)
                nc.tensor.matmul(
                    ps0[:, 0:N].rearrange("p (b s) -> p b s", s=S),
                    wq_t[0:E, :], qa_bc, start=False, stop=True)
                nc.tensor.matmul(ps0[:, 512:512 + N], wk_t[E:128, :], kt[E:128],
                                 start=True, stop=False)
                nc.tensor.matmul(ps0[:, 512:512 + N], wc_t[E:128, :], qk[E:128],
                                 start=False, stop=False)
                nc.tensor.matmul(
                    ps0[:, 512:512 + N].rearrange("p (b s) -> p b s", s=S),
                    wq_t[E:128, :], qb_bc, start=False, stop=True)

                # relu over both groups in one ACT pass (strided PSUM read)
                h1 = mpool.tile([128, 2 * N], dt_mm, tag="h1")
                nc.scalar.activation(
                    h1[:].rearrange("p (c n) -> p c n", c=2),
                    ps0[:].rearrange("p (c n) -> p c n", c=2)[:, :, 0:N],
                    AF.Relu,
                    bias=b0_t,
                )

                # ---- layer 1: h2 = relu(W1.T h1 + b1) (pair-packed out) ----
                ps1 = psh2pool.tile([128, 512], f32, tag="ps1")
                nc.tensor.matmul(ps1[0:H2, 0:N], w1_t[:], h1[:, 0:N],
                                 start=True, stop=True)
                nc.tensor.matmul(ps1[H2:128, 0:N], w1_t[:], h1[:, N:2 * N],
                                 start=True, stop=True, tile_position=(0, 64))
                h2 = mpool.tile([128, N], dt_mm, tag="h2")
                if p % 2 == 0:
                    nc.scalar.activation(h2[:], ps1[:, 0:N], AF.Relu, bias=b1_t)
                else:
                    nc.vector.tensor_scalar(
                        h2[:], ps1[:, 0:N], scalar1=b1_t, scalar2=0.0,
                        op0=OP.add, op1=OP.max)

                # ---- layer 2: scores (pair-packed, broadcast over 64 parts) ----
                ps2 = psspool.tile([128, 512], f32, tag="ps2")
                nc.tensor.matmul(ps2[0:H2, 0:N], w2_t[0:H2, :], h2[0:H2, :],
                                 start=True, stop=True)
                nc.tensor.matmul(ps2[H2:128, 0:N], w2_t[H2:128, :], h2[H2:128, :],
                                 start=True, stop=True, tile_position=(64, 64))

                # ---- exp (unnormalized softmax numerator) ----
                expm = mpool.tile([128, N], f32, tag="expm")
                nc.scalar.activation(expm[:], ps2[:, 0:N], AF.Exp, bias=b2_t)
                nc.gpsimd.dma_start(em_d[2 * p:2 * p + 2, :],
                                    expm[0:E + 1:E, :])

                # ---- unnormalized out = sum_s expm * k (invalid k cols are 0) --
                outw = mpool.tile([128, N], f32, tag="outw")
                nc.vector.tensor_tensor(outw[:], ktf, expm[:], op=OP.mult)
                nc.vector.tensor_reduce(
                    outT_t[:, TB * p:TB * (p + 1)],
                    outw[:].rearrange("p (b s) -> p b s", s=S),
                    axis=mybir.AxisListType.X, op=OP.add)

            nc.sync.dma_start(outT_d[:], outT_t[:])

    nc.compile()
    return nc


def _get_program():
    if "nc" not in _PROG:
        _PROG["nc"] = _build_program()
    return _PROG["nc"]


def _np_mm_dtype():
    if MM_DTYPE == "bfloat16":
        import ml_dtypes
        return np.dtype(ml_dtypes.bfloat16)
    return np.dtype(np.float32)


def kernel(query, keys, keys_length, W0, b0, W1, b1, W2, b2):
    from concourse.bass_utils import run_bass_kernel_spmd

    query = np.asarray(query, dtype=np.float32)
    keys = np.asarray(keys, dtype=np.float32)
    keys_length = np.asarray(keys_length)
    W0 = np.asarray(W0, dtype=np.float32)
    b0 = np.asarray(b0, dtype=np.float32)
    W1 = np.asarray(W1, dtype=np.float32)
    b1 = np.asarray(b1, dtype=np.float32)
    W2 = np.asarray(W2, dtype=np.float32)
    b2 = np.asarray(b2, dtype=np.float32)
    npdt = _np_mm_dtype()

    # ---- host-side weight folding (exact algebra) ----
    Wq = W0[0:E] + W0[3 * E:4 * E]
    Wkk = W0[E:2 * E] - W0[3 * E:4 * E]
    Wc = W0[2 * E:3 * E]
    wq2 = np.concatenate([Wq, Wq], 0)
    wk2 = np.concatenate([Wkk, Wkk], 0)
    wc2 = np.concatenate([Wc, Wc], 0)
    w1p = np.zeros((128, H2), np.float32)
    w1p[:] = W1
    w2r = np.repeat(W2, H2, axis=1)
    w2r2 = np.concatenate([w2r, w2r], 0)
    wpack = np.ascontiguousarray(np.concatenate(
        [wq2, wk2, wc2, w1p, w2r2], axis=1).astype(npdt))
    bpack = np.zeros((128, 3), np.float32)
    bpack[:, 0] = b0
    bpack[:, 1] = np.concatenate([b1, b1])
    bpack[:, 2] = float(b2.reshape(-1)[0])

    mask_full = (np.arange(S)[None, :] < keys_length[:, None])      # [B,S]

    nc = _get_program()

    in_maps = []
    for c in range(NCORES):
        kc = keys[c * BC:(c + 1) * BC] * mask_full[c * BC:(c + 1) * BC, :, None]
        # [1024,100,64] -> feature-major pair-packed [128, NP*400]
        kt = kc.transpose(2, 0, 1).reshape(E, BC * S)               # [64, 102400]
        kTF = np.ascontiguousarray(
            kt.reshape(E, NP, 2, N).transpose(2, 0, 1, 3).reshape(128, NP * N))
        kTP = kTF.astype(npdt)
        qc = query[c * BC:(c + 1) * BC]
        qlP = np.ascontiguousarray(
            qc.reshape(NP, 2, TB, E).transpose(0, 1, 3, 2).reshape(NP, 128, TB)
        ).astype(npdt)
        in_maps.append({"kTP": kTP, "kTF": kTF, "qlP": qlP,
                        "wpack": wpack, "bpack": bpack})

    bkr = run_bass_kernel_spmd(nc, in_maps, list(range(NCORES)))
    _PROG["last_results"] = bkr
    res = bkr.results

    out = np.empty((B, E), dtype=np.float32)
    attn = np.empty((B, S), dtype=np.float32)
    for c in range(NCORES):
        em = res[c]["em2"].reshape(BC, S).astype(np.float64)
        m = mask_full[c * BC:(c + 1) * BC]
        em = em * m
        sums = em.sum(1, keepdims=True)                              # [BC,1]
        attn[c * BC:(c + 1) * BC] = (em / sums).astype(np.float32)
        oT = res[c]["outT"]                                          # [128, 512]
        o = oT.reshape(2, E, NP, TB).transpose(2, 0, 3, 1).reshape(BC, E)
        out[c * BC:(c + 1) * BC] = (o / sums).astype(np.float32)
    return out, attn


# revision 32
# speedup vs baseline: 1.1618x; 1.1618x over previous
"""Trainium2 Bass kernel for nn_AttentionLayer (DIN-style attention scoring MLP).

Math (per batch b, key position s):
    feats = [q, k, q*k, q-k]                       # [4E] = 256
    h1 = relu(feats @ W0 + b0)                     # 128
    h2 = relu(h1 @ W1 + b1)                        # 64
    score = h2 @ W2 + b2                           # scalar
    attn = softmax_s(score masked to s < len[b])
    out = sum_s attn[s] * k[s]                     # [E]

Host-side algebra (exact):
    W0 rows: [0:64]=Wq(q), [64:128]=Wkk(k), [128:192]=Wc(q*k), [192:256]=Wd(q-k)
    h1 = relu(q@(Wq+Wd) + k@(Wkk-Wd) + (q*k)@Wc + b0)
  Invalid key columns (s >= len[b]) are zeroed on the host, so the device's
  unnormalized contraction sum_s exp(score)*k skips them exactly; the softmax
  normalization (divide by masked sum) happens on the host during the gather.

Device layout: feature-major ("transposed") tiles, batch-pair packing.
  Each pair p handles 8 batches = 2 groups (A=batches 8p..8p+3 in
  partitions 0..63, B=batches 8p+4..8p+7 in partitions 64..127).
  N = 4 batches * 100 positions = 400 columns per group.
"""

import numpy as np

B, S, E = 8192, 100, 64
H1, H2 = 128, 64
NCORES = 8
BC = B // NCORES          # 1024 batches per core
TB = 4                    # batches per group
N = TB * S                # 400 columns per matmul
NP = BC // (2 * TB)       # 128 pairs per core

MM_DTYPE = "bfloat16"     # "bfloat16" | "float32r" | "float32"

_PROG = {}


def _build_program():
    import concourse.bacc as bacc
    import concourse.tile as tile
    import concourse.mybir as mybir

    f32 = mybir.dt.float32
    dt_mm = getattr(mybir.dt, MM_DTYPE)
    nc = bacc.Bacc(
        "TRN2", target_bir_lowering=False, debug=False, num_devices=NCORES)

    # ---- DRAM I/O ----
    # kTP pair-packed: [128, NP*N]; rows 0:64 = keys^T of group 2p,
    # rows 64:128 = keys^T of group 2p+1, at columns p*N:(p+1)*N.
    kT_d = nc.declare_dram_parameter("kTP", [128, NP * N], dt_mm, isOutput=False)
    kTF_d = nc.declare_dram_parameter("kTF", [128, NP * N], f32, isOutput=False)
    ql_d = nc.declare_dram_parameter("qlP", [NP, 128, TB], dt_mm, isOutput=False)
    # weights packed [128, 3*H1 + 2*H2] in matmul dtype
    NW = 3 * H1 + 2 * H2
    w_d = nc.declare_dram_parameter("wpack", [128, NW], dt_mm, isOutput=False)
    b_d = nc.declare_dram_parameter("bpack", [128, 3], f32, isOutput=False)

    em_d = nc.declare_dram_parameter("em2", [2 * NP, N], f32, isOutput=True)
    outT_d = nc.declare_dram_parameter("outT", [128, BC // 2], f32, isOutput=True)

    AF = mybir.ActivationFunctionType
    OP = mybir.AluOpType

    with tile.TileContext(nc) as tc:
        with (
            tc.tile_pool(name="consts", bufs=1) as cpool,
            tc.tile_pool(name="kin", bufs=4) as kpool,
            tc.tile_pool(name="qlp", bufs=4) as qlpool,
            tc.tile_pool(name="mid", bufs=4) as mpool,
            tc.tile_pool(name="psl0", bufs=2, space="PSUM") as psl0pool,
            tc.tile_pool(name="psh2", bufs=2, space="PSUM") as psh2pool,
            tc.tile_pool(name="pss", bufs=2, space="PSUM") as psspool,
            tc.tile_pool(name="acc", bufs=1) as apool,
        ):
            w_t = cpool.tile([128, NW], dt_mm, tag="wpack")
            b_t = cpool.tile([128, 3], f32, tag="bpack")
            nc.sync.dma_start(w_t[:], w_d[:])
            nc.sync.dma_start(b_t[:], b_d[:])
            wq_t = w_t[:, 0:H1]
            wk_t = w_t[:, H1:2 * H1]
            wc_t = w_t[:, 2 * H1:3 * H1]
            w1_t = w_t[:, 3 * H1:3 * H1 + H2]
            w2_t = w_t[:, 3 * H1 + H2:3 * H1 + 2 * H2]
            b0_t = b_t[:, 0:1]
            b1_t = b_t[:, 1:2]
            b2_t = b_t[:, 2:3]

            outT_t = apool.tile([128, BC // 2], f32, tag="outT")

            KB = min(4, NP)    # pairs per keys-block DMA / qk op
            QB = min(8, NP)    # pairs per ql block DMA
            ktb = None
            qkb = None
            qlb = None
            for p in range(NP):
                # ---- block loads: keys (4 pairs), q (8 pairs) ----
                if p % QB == 0:
                    qlb = qlpool.tile([128, QB * TB], dt_mm, tag="ql")
                    nc.sync.dma_start(
                        qlb[:].rearrange("p (q t) -> p q t", q=QB),
                        ql_d[p:p + QB].rearrange("q p t -> p q t"))
                if p % KB == 0:
                    ktb = kpool.tile([128, KB * N], dt_mm, tag="kt")
                    nc.sync.dma_start(ktb[:], kT_d[:, p * N:(p + KB) * N])
                    ktfb = kpool.tile([128, KB * N], f32, tag="ktf")
                    nc.sync.dma_start(ktfb[:], kTF_d[:, p * N:(p + KB) * N])
                    qkb = kpool.tile([128, KB * N], dt_mm, tag="qk")
                    q_bc = (qlb[:, (p % QB) * TB:(p % QB + KB) * TB]
                            .unsqueeze(2).broadcast_to((128, KB * TB, S)))
                    nc.vector.tensor_tensor(
                        qkb[:].rearrange("p (b s) -> p b s", s=S),
                        ktb[:].rearrange("p (b s) -> p b s", s=S),
                        q_bc,
                        op=OP.mult,
                    )
                kt = ktb[:, (p % KB) * N:(p % KB + 1) * N]
                ktf = ktfb[:, (p % KB) * N:(p % KB + 1) * N]
                qk = qkb[:, (p % KB) * N:(p % KB + 1) * N]
                ql_t = qlb[:, (p % QB) * TB:(p % QB + 1) * TB]

                # ---- layer 0: h1 = relu(Wk.T k + Wc.T qk + Wq.T q + b0) ----
                ps0 = psl0pool.tile([128, 1024], f32, tag="ps0")
                qa_bc = ql_t[0:E].unsqueeze(2).broadcast_to((E, TB, S))
                qb_bc = ql_t[E:128].unsqueeze(2).broadcast_to((E, TB, S))
                nc.tensor.matmul(ps0[:, 0:N], wk_t[0:E, :], kt[0:E],
                                 start=True, stop=False)
                nc.tensor.matmul(ps0[:, 0:N], wc_t[0:E, :], qk[0:E],
                                 start=False, stop=False)
                nc.tensor.matmul(
                    ps0[:, 0:N].rearrange("p (b s) -> p b s", s=S),
                    wq_t[0:E, :], qa_bc, start=False, stop=True)
                nc.tensor.matmul(ps0[:, 512:512 + N], wk_t[E:128, :], kt[E:128],
                                 start=True, stop=False)
                nc.tensor.matmul(ps0[:, 512:512 + N], wc_t[E:128, :], qk[E:128],
                                 start=False, stop=False)
                nc.tensor.matmul(
                    ps0[:, 512:512 + N].rearrange("p (b s) -> p b s", s=S),
                    wq_t[E:128, :], qb_bc, start=False, stop=True)

                # relu over both groups in one ACT pass (strided PSUM read)
                h1 = mpool.tile([128, 2 * N], dt_mm, tag="h1")
                nc.scalar.activation(
                    h1[:].rearrange("p (c n) -> p c n", c=2),
                    ps0[:].rearrange("p (c n) -> p c n", c=2)[:, :, 0:N],
                    AF.Relu,
                    bias=b0_t,
                )

                # ---- layer 1: h2 = relu(W1.T h1 + b1) (pair-packed out) ----
                ps1 = psh2pool.tile([128, 512], f32, tag="ps1")
                nc.tensor.matmul(ps1[0:H2, 0:N], w1_t[:], h1[:, 0:N],
                                 start=True, stop=True)
                nc.tensor.matmul(ps1[H2:128, 0:N], w1_t[:], h1[:, N:2 * N],
                                 start=True, stop=True, tile_position=(0, 64))
                h2 = mpool.tile([128, N], dt_mm, tag="h2")
                if p % 2 == 0:
                    nc.scalar.activation(h2[:], ps1[:, 0:N], AF.Relu, bias=b1_t)
                else:
                    nc.vector.tensor_scalar(
                        h2[:], ps1[:, 0:N], scalar1=b1_t, scalar2=0.0,
                        op0=OP.add, op1=OP.max)

                # ---- layer 2: scores (pair-packed, broadcast over 64 parts) ----
                ps2 = psspool.tile([128, 512], f32, tag="ps2")
                nc.tensor.matmul(ps2[0:H2, 0:N], w2_t[0:H2, :], h2[0:H2, :],
                                 start=True, stop=True)
                nc.tensor.matmul(ps2[H2:128, 0:N], w2_t[H2:128, :], h2[H2:128, :],
                                 start=True, stop=True, tile_position=(64, 64))

                # ---- exp (unnormalized softmax numerator) ----
                expm = mpool.tile([128, N], f32, tag="expm")
                nc.scalar.activation(expm[:], ps2[:, 0:N], AF.Exp, bias=b2_t)
                nc.gpsimd.dma_start(em_d[2 * p:2 * p + 2, :],
                                    expm[0:E + 1:E, :])

                # ---- unnormalized out = sum_s expm * k (invalid k cols are 0) --
                outw = mpool.tile([128, N], f32, tag="outw")
                nc.vector.tensor_tensor(outw[:], ktf, expm[:], op=OP.mult)
                nc.vector.tensor_reduce(
                    outT_t[:, TB * p:TB * (p + 1)],
                    outw[:].rearrange("p (b s) -> p b s", s=S),
                    axis=mybir.AxisListType.X, op=OP.add)

            nc.sync.dma_start(outT_d[:], outT_t[:])

    nc.compile()
    return nc


def _get_program():
    if "nc" not in _PROG:
        _PROG["nc"] = _build_program()
    return _PROG["nc"]


def _np_mm_dtype():
    if MM_DTYPE == "bfloat16":
        import ml_dtypes
        return np.dtype(ml_dtypes.bfloat16)
    return np.dtype(np.float32)


def kernel(query, keys, keys_length, W0, b0, W1, b1, W2, b2):
    from concourse.bass_utils import run_bass_kernel_spmd

    query = np.asarray(query, dtype=np.float32)
    keys = np.asarray(keys, dtype=np.float32)
    keys_length = np.asarray(keys_length)
    W0 = np.asarray(W0, dtype=np.float32)
    b0 = np.asarray(b0, dtype=np.float32)
    W1 = np.asarray(W1, dtype=np.float32)
    b1 = np.asarray(b1, dtype=np.float32)
    W2 = np.asarray(W2, dtype=np.float32)
    b2 = np.asarray(b2, dtype=np.float32)
    npdt = _np_mm_dtype()

    # ---- host-side weight folding (exact algebra) ----
    Wq = W0[0:E] + W0[3 * E:4 * E]
    Wkk = W0[E:2 * E] - W0[3 * E:4 * E]
    Wc = W0[2 * E:3 * E]
    wq2 = np.concatenate([Wq, Wq], 0)
    wk2 = np.concatenate([Wkk, Wkk], 0)
    wc2 = np.concatenate([Wc, Wc], 0)
    w1p = np.zeros((128, H2), np.float32)
    w1p[:] = W1
    w2r = np.repeat(W2, H2, axis=1)
    w2r2 = np.concatenate([w2r, w2r], 0)
    wpack = np.ascontiguousarray(np.concatenate(
        [wq2, wk2, wc2, w1p, w2r2], axis=1).astype(npdt))
    bpack = np.zeros((128, 3), np.float32)
    bpack[:, 0] = b0
    bpack[:, 1] = np.concatenate([b1, b1])
    bpack[:, 2] = float(b2.reshape(-1)[0])

    mask_full = (np.arange(S)[None, :] < keys_length[:, None])      # [B,S]

    nc = _get_program()

    in_maps = []
    for c in range(NCORES):
        kc = keys[c * BC:(c + 1) * BC] * mask_full[c * BC:(c + 1) * BC, :, None]
        # [1024,100,64] -> feature-major pair-packed [128, NP*400]
        kt = kc.transpose(2, 0, 1).reshape(E, BC * S)               # [64, 102400]
        kTF = np.ascontiguousarray(
            kt.reshape(E, NP, 2, N).transpose(2, 0, 1, 3).reshape(128, NP * N))
        kTP = kTF.astype(npdt)
        qc = query[c * BC:(c + 1) * BC]
        qlP = np.ascontiguousarray(
            qc.reshape(NP, 2, TB, E).transpose(0, 1, 3, 2).reshape(NP, 128, TB)
        ).astype(npdt)
        in_maps.append({"kTP": kTP, "kTF": kTF, "qlP": qlP,
                        "wpack": wpack, "bpack": bpack})

    bkr = run_bass_kernel_spmd(nc, in_maps, list(range(NCORES)))
    _PROG["last_results"] = bkr
    res = bkr.results

    out = np.empty((B, E), dtype=np.float32)
    attn = np.empty((B, S), dtype=np.float32)
    for c in range(NCORES):
        em = res[c]["em2"].reshape(BC, S).astype(np.float64)
        m = mask_full[c * BC:(c + 1) * BC]
        em = em * m
        sums = em.sum(1, keepdims=True)                              # [BC,1]
        attn[c * BC:(c + 1) * BC] = (em / sums).astype(np.float32)
        oT = res[c]["outT"]                                          # [128, 512]
        o = oT.reshape(2, E, NP, TB).transpose(2, 0, 3, 1).reshape(BC, E)
        out[c * BC:(c + 1) * BC] = (o / sums).astype(np.float32)
    return out, attn


# revision 33
# speedup vs baseline: 1.1907x; 1.0249x over previous
"""Trainium2 Bass kernel for nn_AttentionLayer (DIN-style attention scoring MLP).

Math (per batch b, key position s):
    feats = [q, k, q*k, q-k]                       # [4E] = 256
    h1 = relu(feats @ W0 + b0)                     # 128
    h2 = relu(h1 @ W1 + b1)                        # 64
    score = h2 @ W2 + b2                           # scalar
    attn = softmax_s(score masked to s < len[b])
    out = sum_s attn[s] * k[s]                     # [E]

Host-side algebra (exact):
    W0 rows: [0:64]=Wq(q), [64:128]=Wkk(k), [128:192]=Wc(q*k), [192:256]=Wd(q-k)
    h1 = relu(q@(Wq+Wd) + k@(Wkk-Wd) + (q*k)@Wc + b0)
  Invalid key columns (s >= len[b]) are zeroed on the host, so the device's
  unnormalized contraction sum_s exp(score)*k skips them exactly; the softmax
  normalization (divide by masked sum) happens on the host during the gather.

Device layout: feature-major ("transposed") tiles, batch-pair packing.
  Pair p covers 2*tb batches: group A (tb batches) in partitions 0..63,
  group B in partitions 64..127; N = tb*100 columns per group.
  Ragged schedule per core: 100 pairs of tb=5 + 3 pairs of tb=4 = 1024.
"""

import numpy as np

B, S, E = 8192, 100, 64
H1, H2 = 128, 64
NCORES = 8
BC = B // NCORES          # 1024 batches per core

SCHED = [5] * 100 + [4] * 3          # tb per pair; sum*2 == BC
NP2 = len(SCHED)                     # 103 pairs
HTOT = sum(SCHED)                    # 512 half-columns
NMAX = max(SCHED) * S                # 500

MM_DTYPE = "bfloat16"     # "bfloat16" | "float32r" | "float32"

_PROG = {}


def _schedule():
    out = []
    h = 0
    for tb in SCHED:
        out.append((tb, h))
        h += tb
    assert h == HTOT
    return out


def _build_program():
    import concourse.bacc as bacc
    import concourse.tile as tile
    import concourse.mybir as mybir

    f32 = mybir.dt.float32
    dt_mm = getattr(mybir.dt, MM_DTYPE)
    nc = bacc.Bacc(
        "TRN2", target_bir_lowering=False, debug=False, num_devices=NCORES)

    # ---- DRAM I/O ----
    kT_d = nc.declare_dram_parameter("kTP", [128, HTOT * S], dt_mm, isOutput=False)
    kTF_d = nc.declare_dram_parameter("kTF", [128, HTOT * S], f32, isOutput=False)
    ql_d = nc.declare_dram_parameter("qlF", [128, HTOT], dt_mm, isOutput=False)
    NW = 3 * H1 + 2 * H2
    w_d = nc.declare_dram_parameter("wpack", [128, NW], dt_mm, isOutput=False)
    b_d = nc.declare_dram_parameter("bpack", [128, 3], f32, isOutput=False)

    em_d = nc.declare_dram_parameter("em2", [NP2, 2, NMAX], f32, isOutput=True)
    outT_d = nc.declare_dram_parameter("outT", [128, HTOT], f32, isOutput=True)

    AF = mybir.ActivationFunctionType
    OP = mybir.AluOpType
    sched = _schedule()

    # keys-block boundaries: greedy pack pairs while block width <= 4*500 cols
    blocks = []
    cur = []
    width = 0
    for idx, (tb, h) in enumerate(sched):
        if cur and width + tb * S > 2000:
            blocks.append(cur)
            cur, width = [], 0
        cur.append(idx)
        width += tb * S
    blocks.append(cur)
    blk_of = {}
    for bi, idxs in enumerate(blocks):
        for idx in idxs:
            blk_of[idx] = bi

    with tile.TileContext(nc) as tc:
        with (
            tc.tile_pool(name="consts", bufs=1) as cpool,
            tc.tile_pool(name="kin", bufs=4) as kpool,
            tc.tile_pool(name="mid", bufs=4) as mpool,
            tc.tile_pool(name="psl0", bufs=2, space="PSUM") as psl0pool,
            tc.tile_pool(name="psh2", bufs=2, space="PSUM") as psh2pool,
            tc.tile_pool(name="pss", bufs=2, space="PSUM") as psspool,
            tc.tile_pool(name="acc", bufs=1) as apool,
        ):
            w_t = cpool.tile([128, NW], dt_mm, tag="wpack")
            b_t = cpool.tile([128, 3], f32, tag="bpack")
            qlF_t = cpool.tile([128, HTOT], dt_mm, tag="qlF")
            nc.sync.dma_start(w_t[:], w_d[:])
            nc.sync.dma_start(b_t[:], b_d[:])
            nc.sync.dma_start(qlF_t[:], ql_d[:])
            wq_t = w_t[:, 0:H1]
            wk_t = w_t[:, H1:2 * H1]
            wc_t = w_t[:, 2 * H1:3 * H1]
            w1_t = w_t[:, 3 * H1:3 * H1 + H2]
            w2_t = w_t[:, 3 * H1 + H2:3 * H1 + 2 * H2]
            b0_t = b_t[:, 0:1]
            b1_t = b_t[:, 1:2]
            b2_t = b_t[:, 2:3]

            outT_t = apool.tile([128, HTOT], f32, tag="outT")
            outT_flushed = 0

            ktb = ktfb = qkb = None
            blk_h0 = 0
            for p, (tb, h) in enumerate(sched):
                NN = tb * S
                # ---- block load of keys + q*k ----
                if blk_of[p] != blk_of.get(p - 1, -1):
                    idxs = blocks[blk_of[p]]
                    blk_h0 = sched[idxs[0]][1]
                    blk_h1 = sched[idxs[-1]][1] + sched[idxs[-1]][0]
                    bw = (blk_h1 - blk_h0) * S
                    ktb = kpool.tile([128, 2000], dt_mm, tag="kt")
                    nc.sync.dma_start(
                        ktb[:, 0:bw], kT_d[:, blk_h0 * S:blk_h1 * S])
                    ktfb = kpool.tile([128, 2000], f32, tag="ktf")
                    nc.sync.dma_start(
                        ktfb[:, 0:bw], kTF_d[:, blk_h0 * S:blk_h1 * S])
                    qkb = kpool.tile([128, 2000], dt_mm, tag="qk")
                    q_bc = (qlF_t[:, blk_h0:blk_h1]
                            .unsqueeze(2).broadcast_to((128, blk_h1 - blk_h0, S)))
                    nc.vector.tensor_tensor(
                        qkb[:, 0:bw].rearrange("p (b s) -> p b s", s=S),
                        ktb[:, 0:bw].rearrange("p (b s) -> p b s", s=S),
                        q_bc,
                        op=OP.mult,
                    )
                co = (h - blk_h0) * S
                kt = ktb[:, co:co + NN]
                ktf = ktfb[:, co:co + NN]
                qk = qkb[:, co:co + NN]
                ql_t = qlF_t[:, h:h + tb]

                # ---- layer 0 ----
                ps0 = psl0pool.tile([128, 1024], f32, tag="ps0")
                qa_bc = ql_t[0:E].unsqueeze(2).broadcast_to((E, tb, S))
                qb_bc = ql_t[E:128].unsqueeze(2).broadcast_to((E, tb, S))
                nc.tensor.matmul(ps0[:, 0:NN], wk_t[0:E, :], kt[0:E],
                                 start=True, stop=False)
                nc.tensor.matmul(ps0[:, 0:NN], wc_t[0:E, :], qk[0:E],
                                 start=False, stop=False)
                nc.tensor.matmul(
                    ps0[:, 0:NN].rearrange("p (b s) -> p b s", s=S),
                    wq_t[0:E, :], qa_bc, start=False, stop=True)
                nc.tensor.matmul(ps0[:, 512:512 + NN], wk_t[E:128, :], kt[E:128],
                                 start=True, stop=False)
                nc.tensor.matmul(ps0[:, 512:512 + NN], wc_t[E:128, :], qk[E:128],
                                 start=False, stop=False)
                nc.tensor.matmul(
                    ps0[:, 512:512 + NN].rearrange("p (b s) -> p b s", s=S),
                    wq_t[E:128, :], qb_bc, start=False, stop=True)

                h1 = mpool.tile([128, 2 * NMAX], dt_mm, tag="h1")
                nc.scalar.activation(
                    h1[:, 0:2 * NN].rearrange("p (c n) -> p c n", c=2),
                    ps0[:].rearrange("p (c n) -> p c n", c=2)[:, :, 0:NN],
                    AF.Relu,
                    bias=b0_t,
                )

                # ---- layer 1 ----
                ps1 = psh2pool.tile([128, 512], f32, tag="ps1")
                nc.tensor.matmul(ps1[0:H2, 0:NN], w1_t[:], h1[:, 0:NN],
                                 start=True, stop=True)
                nc.tensor.matmul(ps1[H2:128, 0:NN], w1_t[:], h1[:, NN:2 * NN],
                                 start=True, stop=True, tile_position=(0, 64))
                h2 = mpool.tile([128, NMAX], dt_mm, tag="h2")
                if p % 2 == 0:
                    nc.scalar.activation(h2[:, 0:NN], ps1[:, 0:NN],
                                         AF.Relu, bias=b1_t)
                else:
                    nc.vector.tensor_scalar(
                        h2[:, 0:NN], ps1[:, 0:NN], scalar1=b1_t, scalar2=0.0,
                        op0=OP.add, op1=OP.max)

                # ---- layer 2 ----
                ps2 = psspool.tile([128, 512], f32, tag="ps2")
                nc.tensor.matmul(ps2[0:H2, 0:NN], w2_t[0:H2, :], h2[0:H2, 0:NN],
                                 start=True, stop=True)
                nc.tensor.matmul(ps2[H2:128, 0:NN], w2_t[H2:128, :],
                                 h2[H2:128, 0:NN],
                                 start=True, stop=True, tile_position=(64, 64))

                # ---- exp + ship numerators ----
                expm = mpool.tile([128, NMAX], f32, tag="expm")
                nc.scalar.activation(expm[:, 0:NN], ps2[:, 0:NN], AF.Exp,
                                     bias=b2_t)
                nc.gpsimd.dma_start(em_d[p, :, 0:NN], expm[0:E + 1:E, 0:NN])

                # ---- unnormalized out = sum_s expm * k ----
                outw = mpool.tile([128, NMAX], f32, tag="outw")
                nc.vector.tensor_tensor(outw[:, 0:NN], ktf, expm[:, 0:NN],
                                        op=OP.mult)
                nc.vector.tensor_reduce(
                    outT_t[:, h:h + tb],
                    outw[:, 0:NN].rearrange("p (b s) -> p b s", s=S),
                    axis=mybir.AxisListType.X, op=OP.add)

                # flush finished outT columns periodically (shrinks drain tail)
                if p % 24 == 23:
                    nc.sync.dma_start(outT_d[:, outT_flushed:h + tb],
                                      outT_t[:, outT_flushed:h + tb])
                    outT_flushed = h + tb

            if outT_flushed < HTOT:
                nc.sync.dma_start(outT_d[:, outT_flushed:HTOT],
                                  outT_t[:, outT_flushed:HTOT])

    nc.compile()
    return nc


def _get_program():
    if "nc" not in _PROG:
        _PROG["nc"] = _build_program()
    return _PROG["nc"]


def _np_mm_dtype():
    if MM_DTYPE == "bfloat16":
        import ml_dtypes
        return np.dtype(ml_dtypes.bfloat16)
    return np.dtype(np.float32)


def kernel(query, keys, keys_length, W0, b0, W1, b1, W2, b2):
    from concourse.bass_utils import run_bass_kernel_spmd

    query = np.asarray(query, dtype=np.float32)
    keys = np.asarray(keys, dtype=np.float32)
    keys_length = np.asarray(keys_length)
    W0 = np.asarray(W0, dtype=np.float32)
    b0 = np.asarray(b0, dtype=np.float32)
    W1 = np.asarray(W1, dtype=np.float32)
    b1 = np.asarray(b1, dtype=np.float32)
    W2 = np.asarray(W2, dtype=np.float32)
    b2 = np.asarray(b2, dtype=np.float32)
    npdt = _np_mm_dtype()
    sched = _schedule()

    Wq = W0[0:E] + W0[3 * E:4 * E]
    Wkk = W0[E:2 * E] - W0[3 * E:4 * E]
    Wc = W0[2 * E:3 * E]
    wq2 = np.concatenate([Wq, Wq], 0)
    wk2 = np.concatenate([Wkk, Wkk], 0)
    wc2 = np.concatenate([Wc, Wc], 0)
    w1p = np.zeros((128, H2), np.float32)
    w1p[:] = W1
    w2r = np.repeat(W2, H2, axis=1)
    w2r2 = np.concatenate([w2r, w2r], 0)
    wpack = np.ascontiguousarray(np.concatenate(
        [wq2, wk2, wc2, w1p, w2r2], axis=1).astype(npdt))
    bpack = np.zeros((128, 3), np.float32)
    bpack[:, 0] = b0
    bpack[:, 1] = np.concatenate([b1, b1])
    bpack[:, 2] = float(b2.reshape(-1)[0])

    mask_full = (np.arange(S)[None, :] < keys_length[:, None])      # [B,S]

    nc = _get_program()

    in_maps = []
    for c in range(NCORES):
        kc = keys[c * BC:(c + 1) * BC] * mask_full[c * BC:(c + 1) * BC, :, None]
        kt_half = kc.transpose(2, 0, 1).reshape(E, BC * S)          # [64, BC*S]
        # pair-packed: rows 0:64 = group A halves, 64:128 = group B
        kTF = np.empty((128, HTOT * S), np.float32)
        qc = query[c * BC:(c + 1) * BC]
        qlF = np.empty((128, HTOT), np.float32)
        for tb, h in sched:
            b0i = 2 * h
            kTF[0:E, h * S:(h + tb) * S] = \
                kt_half[:, b0i * S:(b0i + tb) * S]
            kTF[E:128, h * S:(h + tb) * S] = \
                kt_half[:, (b0i + tb) * S:(b0i + 2 * tb) * S]
            qlF[0:E, h:h + tb] = qc[b0i:b0i + tb].T
            qlF[E:128, h:h + tb] = qc[b0i + tb:b0i + 2 * tb].T
        in_maps.append({
            "kTP": kTF.astype(npdt), "kTF": kTF,
            "qlF": qlF.astype(npdt),
            "wpack": wpack, "bpack": bpack,
        })

    bkr = run_bass_kernel_spmd(nc, in_maps, list(range(NCORES)))
    _PROG["last_results"] = bkr
    res = bkr.results

    out = np.empty((B, E), dtype=np.float32)
    attn = np.empty((B, S), dtype=np.float32)
    for c in range(NCORES):
        em2 = res[c]["em2"]                                          # [NP2,2,NMAX]
        oT = res[c]["outT"]                                          # [128,HTOT]
        ob = c * BC
        for p, (tb, h) in enumerate(sched):
            b0i = ob + 2 * h
            em = np.concatenate(
                [em2[p, 0, 0:tb * S].reshape(tb, S),
                 em2[p, 1, 0:tb * S].reshape(tb, S)], 0).astype(np.float64)
            m = mask_full[b0i:b0i + 2 * tb]
            em = em * m
            sums = em.sum(1, keepdims=True)
            attn[b0i:b0i + 2 * tb] = em / sums
            o = np.concatenate([oT[0:E, h:h + tb].T, oT[E:128, h:h + tb].T], 0)
            out[b0i:b0i + 2 * tb] = o / sums
    return out, attn


# revision 37
# speedup vs baseline: 1.2112x; 1.0172x over previous
"""Trainium2 Bass kernel for nn_AttentionLayer (DIN-style attention scoring MLP).

Math (per batch b, key position s):
    feats = [q, k, q*k, q-k]                       # [4E] = 256
    h1 = relu(feats @ W0 + b0)                     # 128
    h2 = relu(h1 @ W1 + b1)                        # 64
    score = h2 @ W2 + b2                           # scalar
    attn = softmax_s(score masked to s < len[b])
    out = sum_s attn[s] * k[s]                     # [E]

Host-side algebra (exact):
    W0 rows: [0:64]=Wq(q), [64:128]=Wkk(k), [128:192]=Wc(q*k), [192:256]=Wd(q-k)
    h1 = relu(q@(Wq+Wd) + k@(Wkk-Wd) + (q*k)@Wc + b0)
  Invalid key columns (s >= len[b]) are zeroed on the host, so the device's
  unnormalized contraction sum_s exp(score)*k skips them exactly; the softmax
  normalization (divide by masked sum) happens on the host during the gather.

Device layout: feature-major ("transposed") tiles, batch-pair packing.
  Pair p covers 2*tb batches: group A (tb batches) in partitions 0..63,
  group B in partitions 64..127; N = tb*100 columns per group.
  Ragged schedule per core: 100 pairs of tb=5 + 3 pairs of tb=4 = 1024.
"""

import numpy as np

B, S, E = 8192, 100, 64
H1, H2 = 128, 64
NCORES = 8
BC = B // NCORES          # 1024 batches per core

SCHED = [5] * 100 + [4] * 3          # tb per pair; sum*2 == BC
NP2 = len(SCHED)                     # 103 pairs
HTOT = sum(SCHED)                    # 512 half-columns
NMAX = max(SCHED) * S                # 500

MM_DTYPE = "bfloat16"     # "bfloat16" | "float32r" | "float32"

_PROG = {}


def _schedule():
    out = []
    h = 0
    for tb in SCHED:
        out.append((tb, h))
        h += tb
    assert h == HTOT
    return out


def _build_program():
    import concourse.bacc as bacc
    import concourse.tile as tile
    import concourse.mybir as mybir

    f32 = mybir.dt.float32
    dt_mm = getattr(mybir.dt, MM_DTYPE)
    nc = bacc.Bacc(
        "TRN2", target_bir_lowering=False, debug=False, num_devices=NCORES)

    # ---- DRAM I/O ----
    kT_d = nc.declare_dram_parameter("kTP", [128, HTOT * S], dt_mm, isOutput=False)
    kTF_d = nc.declare_dram_parameter("kTF", [128, HTOT * S], f32, isOutput=False)
    ql_d = nc.declare_dram_parameter("qlF", [128, HTOT], dt_mm, isOutput=False)
    qb_d = nc.declare_dram_parameter("qbF", [128, HTOT * S], dt_mm, isOutput=False)
    NW = 3 * H1 + 2 * H2
    w_d = nc.declare_dram_parameter("wpack", [128, NW], dt_mm, isOutput=False)
    b_d = nc.declare_dram_parameter("bpack", [128, 3], f32, isOutput=False)

    em_d = nc.declare_dram_parameter("em2", [NP2, 2, NMAX], f32, isOutput=True)
    outT_d = nc.declare_dram_parameter("outT", [128, HTOT], f32, isOutput=True)

    AF = mybir.ActivationFunctionType
    OP = mybir.AluOpType
    sched = _schedule()

    # keys-block boundaries: greedy pack pairs while block width <= 4*500 cols
    blocks = []
    cur = []
    width = 0
    for idx, (tb, h) in enumerate(sched):
        if cur and width + tb * S > 2000:
            blocks.append(cur)
            cur, width = [], 0
        cur.append(idx)
        width += tb * S
    blocks.append(cur)
    blk_of = {}
    for bi, idxs in enumerate(blocks):
        for idx in idxs:
            blk_of[idx] = bi

    with tile.TileContext(nc) as tc:
        with (
            tc.tile_pool(name="consts", bufs=1) as cpool,
            tc.tile_pool(name="kin", bufs=4) as kpool,
            tc.tile_pool(name="mid", bufs=4) as mpool,
            tc.tile_pool(name="psl0", bufs=2, space="PSUM") as psl0pool,
            tc.tile_pool(name="psh2", bufs=2, space="PSUM") as psh2pool,
            tc.tile_pool(name="pss", bufs=2, space="PSUM") as psspool,
            tc.tile_pool(name="acc", bufs=1) as apool,
        ):
            w_t = cpool.tile([128, NW], dt_mm, tag="wpack")
            b_t = cpool.tile([128, 3], f32, tag="bpack")
            qlF_t = cpool.tile([128, HTOT], dt_mm, tag="qlF")
            nc.sync.dma_start(w_t[:], w_d[:])
            nc.sync.dma_start(b_t[:], b_d[:])
            nc.sync.dma_start(qlF_t[:], ql_d[:])
            wq_t = w_t[:, 0:H1]
            wk_t = w_t[:, H1:2 * H1]
            wc_t = w_t[:, 2 * H1:3 * H1]
            w1_t = w_t[:, 3 * H1:3 * H1 + H2]
            w2_t = w_t[:, 3 * H1 + H2:3 * H1 + 2 * H2]
            b0_t = b_t[:, 0:1]
            b1_t = b_t[:, 1:2]
            b2_t = b_t[:, 2:3]

            outT_t = apool.tile([128, HTOT], f32, tag="outT")
            outT_flushed = 0

            ktb = ktfb = qkb = None
            blk_h0 = 0
            for p, (tb, h) in enumerate(sched):
                NN = tb * S
                # ---- block load of keys + q*k ----
                if blk_of[p] != blk_of.get(p - 1, -1):
                    idxs = blocks[blk_of[p]]
                    blk_h0 = sched[idxs[0]][1]
                    blk_h1 = sched[idxs[-1]][1] + sched[idxs[-1]][0]
                    bw = (blk_h1 - blk_h0) * S
                    ktb = kpool.tile([128, 2000], dt_mm, tag="kt")
                    nc.sync.dma_start(
                        ktb[:, 0:bw], kT_d[:, blk_h0 * S:blk_h1 * S])
                    ktfb = kpool.tile([128, 2000], f32, tag="ktf")
                    nc.sync.dma_start(
                        ktfb[:, 0:bw], kTF_d[:, blk_h0 * S:blk_h1 * S])
                    qbb = kpool.tile([128, 2000], dt_mm, tag="qb")
                    nc.sync.dma_start(
                        qbb[:, 0:bw], qb_d[:, blk_h0 * S:blk_h1 * S])
                    qkb = kpool.tile([128, 2000], dt_mm, tag="qk")
                    nc.vector.tensor_tensor(
                        qkb[:, 0:bw], ktb[:, 0:bw], qbb[:, 0:bw], op=OP.mult)
                co = (h - blk_h0) * S
                kt = ktb[:, co:co + NN]
                ktf = ktfb[:, co:co + NN]
                qk = qkb[:, co:co + NN]
                ql_t = qlF_t[:, h:h + tb]

                # ---- layer 0 ----
                ps0 = psl0pool.tile([128, 1024], f32, tag="ps0")
                qa_bc = ql_t[0:E].unsqueeze(2).broadcast_to((E, tb, S))
                qb_bc = ql_t[E:128].unsqueeze(2).broadcast_to((E, tb, S))
                nc.tensor.matmul(ps0[:, 0:NN], wk_t[0:E, :], kt[0:E],
                                 start=True, stop=False)
                nc.tensor.matmul(ps0[:, 0:NN], wc_t[0:E, :], qk[0:E],
                                 start=False, stop=False)
                nc.tensor.matmul(
                    ps0[:, 0:NN].rearrange("p (b s) -> p b s", s=S),
                    wq_t[0:E, :], qa_bc, start=False, stop=True)
                nc.tensor.matmul(ps0[:, 512:512 + NN], wk_t[E:128, :], kt[E:128],
                                 start=True, stop=False)
                nc.tensor.matmul(ps0[:, 512:512 + NN], wc_t[E:128, :], qk[E:128],
                                 start=False, stop=False)
                nc.tensor.matmul(
                    ps0[:, 512:512 + NN].rearrange("p (b s) -> p b s", s=S),
                    wq_t[E:128, :], qb_bc, start=False, stop=True)

                h1 = mpool.tile([128, 2 * NMAX], dt_mm, tag="h1")
                nc.scalar.activation(
                    h1[:, 0:2 * NN].rearrange("p (c n) -> p c n", c=2),
                    ps0[:].rearrange("p (c n) -> p c n", c=2)[:, :, 0:NN],
                    AF.Relu,
                    bias=b0_t,
                )

                # ---- layer 1 ----
                ps1 = psh2pool.tile([128, 512], f32, tag="ps1")
                nc.tensor.matmul(ps1[0:H2, 0:NN], w1_t[:], h1[:, 0:NN],
                                 start=True, stop=True)
                nc.tensor.matmul(ps1[H2:128, 0:NN], w1_t[:], h1[:, NN:2 * NN],
                                 start=True, stop=True, tile_position=(0, 64))
                h2 = mpool.tile([128, NMAX], dt_mm, tag="h2")
                if p % 3 == 0:
                    nc.scalar.activation(h2[:, 0:NN], ps1[:, 0:NN],
                                         AF.Relu, bias=b1_t)
                else:
                    nc.vector.tensor_scalar(
                        h2[:, 0:NN], ps1[:, 0:NN], scalar1=b1_t, scalar2=0.0,
                        op0=OP.add, op1=OP.max)

                # ---- layer 2 ----
                ps2 = psspool.tile([128, 512], f32, tag="ps2")
                nc.tensor.matmul(ps2[0:H2, 0:NN], w2_t[0:H2, :], h2[0:H2, 0:NN],
                                 start=True, stop=True)
                nc.tensor.matmul(ps2[H2:128, 0:NN], w2_t[H2:128, :],
                                 h2[H2:128, 0:NN],
                                 start=True, stop=True, tile_position=(64, 64))

                # ---- exp + ship numerators ----
                expm = mpool.tile([128, NMAX], f32, tag="expm")
                nc.scalar.activation(expm[:, 0:NN], ps2[:, 0:NN], AF.Exp,
                                     bias=b2_t)
                nc.gpsimd.dma_start(em_d[p, :, 0:NN], expm[0:E + 1:E, 0:NN])

                # ---- unnormalized out = sum_s expm * k ----
                outw = mpool.tile([128, NMAX], f32, tag="outw")
                nc.vector.tensor_tensor(outw[:, 0:NN], ktf, expm[:, 0:NN],
                                        op=OP.mult)
                nc.vector.tensor_reduce(
                    outT_t[:, h:h + tb],
                    outw[:, 0:NN].rearrange("p (b s) -> p b s", s=S),
                    axis=mybir.AxisListType.X, op=OP.add)

                # flush finished outT columns periodically (shrinks drain tail)
                if p % 24 == 23:
                    nc.sync.dma_start(outT_d[:, outT_flushed:h + tb],
                                      outT_t[:, outT_flushed:h + tb])
                    outT_flushed = h + tb

            if outT_flushed < HTOT:
                nc.sync.dma_start(outT_d[:, outT_flushed:HTOT],
                                  outT_t[:, outT_flushed:HTOT])

    nc.compile()
    return nc


def _get_program():
    if "nc" not in _PROG:
        _PROG["nc"] = _build_program()
    return _PROG["nc"]


def _np_mm_dtype():
    if MM_DTYPE == "bfloat16":
        import ml_dtypes
        return np.dtype(ml_dtypes.bfloat16)
    return np.dtype(np.float32)


def kernel(query, keys, keys_length, W0, b0, W1, b1, W2, b2):
    from concourse.bass_utils import run_bass_kernel_spmd

    query = np.asarray(query, dtype=np.float32)
    keys = np.asarray(keys, dtype=np.float32)
    keys_length = np.asarray(keys_length)
    W0 = np.asarray(W0, dtype=np.float32)
    b0 = np.asarray(b0, dtype=np.float32)
    W1 = np.asarray(W1, dtype=np.float32)
    b1 = np.asarray(b1, dtype=np.float32)
    W2 = np.asarray(W2, dtype=np.float32)
    b2 = np.asarray(b2, dtype=np.float32)
    npdt = _np_mm_dtype()
    sched = _schedule()

    Wq = W0[0:E] + W0[3 * E:4 * E]
    Wkk = W0[E:2 * E] - W0[3 * E:4 * E]
    Wc = W0[2 * E:3 * E]
    wq2 = np.concatenate([Wq, Wq], 0)
    wk2 = np.concatenate([Wkk, Wkk], 0)
    wc2 = np.concatenate([Wc, Wc], 0)
    w1p = np.zeros((128, H2), np.float32)
    w1p[:] = W1
    w2r = np.repeat(W2, H2, axis=1)
    w2r2 = np.concatenate([w2r, w2r], 0)
    wpack = np.ascontiguousarray(np.concatenate(
        [wq2, wk2, wc2, w1p, w2r2], axis=1).astype(npdt))
    bpack = np.zeros((128, 3), np.float32)
    bpack[:, 0] = b0
    bpack[:, 1] = np.concatenate([b1, b1])
    bpack[:, 2] = float(b2.reshape(-1)[0])

    mask_full = (np.arange(S)[None, :] < keys_length[:, None])      # [B,S]

    nc = _get_program()

    in_maps = []
    for c in range(NCORES):
        kc = keys[c * BC:(c + 1) * BC] * mask_full[c * BC:(c + 1) * BC, :, None]
        kt_half = kc.transpose(2, 0, 1).reshape(E, BC * S)          # [64, BC*S]
        # pair-packed: rows 0:64 = group A halves, 64:128 = group B
        kTF = np.empty((128, HTOT * S), np.float32)
        qc = query[c * BC:(c + 1) * BC]
        qlF = np.empty((128, HTOT), np.float32)
        for tb, h in sched:
            b0i = 2 * h
            kTF[0:E, h * S:(h + tb) * S] = \
                kt_half[:, b0i * S:(b0i + tb) * S]
            kTF[E:128, h * S:(h + tb) * S] = \
                kt_half[:, (b0i + tb) * S:(b0i + 2 * tb) * S]
            qlF[0:E, h:h + tb] = qc[b0i:b0i + tb].T
            qlF[E:128, h:h + tb] = qc[b0i + tb:b0i + 2 * tb].T
        qlFd = qlF.astype(npdt)
        in_maps.append({
            "kTP": kTF.astype(npdt), "kTF": kTF,
            "qlF": qlFd, "qbF": np.repeat(qlFd, S, axis=1),
            "wpack": wpack, "bpack": bpack,
        })

    bkr = run_bass_kernel_spmd(nc, in_maps, list(range(NCORES)))
    _PROG["last_results"] = bkr
    res = bkr.results

    out = np.empty((B, E), dtype=np.float32)
    attn = np.empty((B, S), dtype=np.float32)
    for c in range(NCORES):
        em2 = res[c]["em2"]                                          # [NP2,2,NMAX]
        oT = res[c]["outT"]                                          # [128,HTOT]
        ob = c * BC
        for p, (tb, h) in enumerate(sched):
            b0i = ob + 2 * h
            em = np.concatenate(
                [em2[p, 0, 0:tb * S].reshape(tb, S),
                 em2[p, 1, 0:tb * S].reshape(tb, S)], 0).astype(np.float64)
            m = mask_full[b0i:b0i + 2 * tb]
            em = em * m
            sums = em.sum(1, keepdims=True)
            attn[b0i:b0i + 2 * tb] = em / sums
            o = np.concatenate([oT[0:E, h:h + tb].T, oT[E:128, h:h + tb].T], 0)
            out[b0i:b0i + 2 * tb] = o / sums
    return out, attn
